# revision 5
# baseline (speedup 1.0000x reference)
"""CPA-loss kernel for 8 TRN2 NeuronCores — v2.

Math: for row b with target t, the loss collapses to
    loss[b] = -log( e[b,t] / (dot(s[t,:], e[b,:]) + eps) + eps ),
    e = exp(z)  (max-subtraction skipped; effect ~2e-7 on the mean loss).

v2 strategy vs the 44.2us baseline (trace-driven):
 - The baseline was paced by the Scalar/ACT engine (exp of 1.64M elems/core at
   1 elem/cycle/lane) plus ~7us of gpsimd SWDGE dge_drain dead time and ~8us
   of fixed framework teardown.  Here the exp is SPLIT between the ACT engine
   and the Vector engine via two custom-DVE ops (registered at import):
     op1: bits = int32((f32(t + K) - 2^23) * 2^23)   -> raw bits of 2^round(t)
     op2: out  = (1 + b1 f + b2 f^2) * bitcast_f32(bits),  f = t - round(t)
   Host ships t = z*log2(e) (+log2(a0) on DVE-assigned columns, which folds
   the poly normalization into the data).  The DVE chain's worst-case rel
   error is 1.7e-3, mean-centered to <1e-6 bias on the final scalar (validated
   against the reference: rel err 8.5e-7 at a 50/50 split).
 - Denominator "+eps" rides the matmul: row 100 of the shipped logits is 0
   (exp -> 1.0 exactly on both engines) and row 100 of the candidate matrix is
   eps, so PSUM holds D+eps directly.
 - Epilogue: dsel select via host masks, 1/(D+eps) via the single-instruction
   reciprocal_approx_fast (~51 ULP), sigma = e_t * rec, then ONE final
   Ln(sigma + eps) with accum_out (bias=eps is free on ACT).  No gpsimd DMA.
"""

import sys

import ml_dtypes  # noqa: F401
import numpy as np

for _p in ("/opt/trn_rl_repo",):
    if _p not in sys.path:
        sys.path.append(_p)

import concourse.bass as bass  # noqa: F401
import concourse.tile as tile
from concourse import bacc, mybir
from concourse.bass_utils import run_bass_kernel_spmd
from concourse import dve_ops as _dvo
from concourse.dve_spec import Spec, Src0, Src1, C0, C1, C2, One, lower, _has_src1
from concourse.dve_uop import DveOpSpec

B = 131072
C = 100
CP = C + 1  # +1 ones/eps row
NCORES = 8
RPC = B // NCORES  # 16384 rows per core
BLK = 128
NBLK = RPC // BLK  # 128 blocks per core
GSIZES = [4, 8, 16, 32, 32, 36]  # blocks per DMA/exp chunk
SLICES = 4
SBLK = NBLK // SLICES
EPS = 1e-6

F = np.float32
LOG2E = float(np.log2(np.e))
LN2 = float(np.log(2.0))
KMAGIC = 8388735.0  # 2^23 + 127
P23 = 8388608.0     # 2^23
# mean-centered poly 2^f ~ a0*(1 + b1 f + b2 f^2) on [-0.5, 0.5]
B1C = 0.7031777501106262
B2C = 0.23833733797073364
LA = 0.0005543692115323172  # log2(a0), host-applied to DVE columns

# fraction of each chunk's blocks handled by the ACT engine (rest on DVE)
ACT_FRAC = 0.66


def _act_blocks(gs: int) -> int:
    return max(1, min(gs - 1, int(round(gs * ACT_FRAC))))


TRACE = False  # test.py flips this to get a profiled run
LAST_RESULTS = None  # stash of the last BassKernelResults (for test.py)

_nc_cache = {}
_ops_cache = []


def _f32(x):
    return np.float32(x)


def _ref1(in0, in1, s0, s1, imm2):
    t = in0.astype(F)
    u = (t + _f32(s0)).astype(F)
    return ((u - _f32(s1)) * _f32(imm2)).astype(F)


def _ref2(in0, in1, s0, s1, imm2):
    t = in0.astype(F)
    u = (t + _f32(imm2)).astype(F)
    n = (u - _f32(imm2)).astype(F)
    f = (t - n).astype(F)
    q = ((_f32(s1) * f).astype(F) + _f32(s0)).astype(F)
    q = (q * f).astype(F)
    q = (q + _f32(1.0)).astype(F)
    return (q * in1.astype(F)).astype(F)


def _register_dve_ops():
    """Register the two exp2 custom-DVE ops (idempotent)."""
    global _ops_cache
    if _ops_cache:
        return _ops_cache
    if "EXP2_BITS_ANT" in _dvo._SUB_OPCODE_FOR_NAME:
        by_name = {o.name: o for o in _dvo.OPS}
        _ops_cache = [by_name["EXP2_BITS_ANT"], by_name["EXP2_FIN_ANT"]]
        return _ops_cache

    def mk(name, body, ref):
        opcode = _dvo._CUSTOM_DVE_ROW_BASE + len(_dvo.OPS)
        spec = Spec(body=body, reference=ref)
        shas = {}
        for ver in ("v3", "v4"):
            ds = DveOpSpec(
                name=name, opcode=opcode, uops=lower(spec, ver=ver),
                rd1_en=_has_src1(spec),
            )
            shas[ver] = ds.sha(ver)
        op = _dvo.DveOp(name, spec, subdim=False, uops_sha=shas)
        _dvo.OPS.append(op)
        _dvo._SUB_OPCODE_FOR_NAME[name] = opcode
        _dvo.CUSTOM_DVE_SPECS[name] = op.spec
        return op

    op1 = mk("EXP2_BITS_ANT", ((Src0 + C0) - C1) * C2, _ref1)
    _u = Src0 + C2
    _n = _u - C2
    _fr = Src0 - _n
    _q = ((C1 * _fr) + C0) * _fr + One
    op2 = mk("EXP2_FIN_ANT", _q * Src1, _ref2)
    _ops_cache = [op1, op2]
    return _ops_cache


def _build_nc(m: int, stride: int):
    op1, op2 = _register_dve_ops()
    nc = bacc.Bacc("TRN2", target_bir_lowering=False, debug=False)
    f32 = mybir.dt.float32
    f16 = mybir.dt.float16
    i32 = mybir.dt.int32

    lt_d = nc.declare_dram_parameter("lt", [CP * RPC], f16, isOutput=False)
    vs_d = nc.declare_dram_parameter("vs", [CP, m * NBLK], f16, isOutput=False)
    zt_d = nc.declare_dram_parameter("zt", [BLK, NBLK], f32, isOutput=False)
    w_d = [
        nc.declare_dram_parameter(f"w{i}", [BLK, NBLK], mybir.dt.uint8, isOutput=False)
        for i in range(max(m - 1, 1))
    ]
    out_d = nc.declare_dram_parameter("out", [BLK, 1], f32, isOutput=True)

    gsizes = GSIZES
    assert sum(gsizes) == NBLK

    with tile.TileContext(nc) as tc:
        with (
            tc.tile_pool(name="const", bufs=1) as cpool,
            tc.tile_pool(name="lt", bufs=3) as ltp,
            tc.tile_pool(name="eta", bufs=2) as etap,
            tc.tile_pool(name="etd", bufs=2) as etdp,
            tc.tile_pool(name="bits", bufs=2) as bitp,
            tc.tile_pool(name="fin", bufs=1) as fin,
            tc.tile_pool(name="res", bufs=1, space="PSUM") as resp,
        ):
            def lt_slice(g):
                off = CP * BLK * sum(gsizes[:g])
                n = CP * gsizes[g] * BLK
                return lt_d[off : off + n].rearrange("(j c) -> j c", j=CP)

            # first logits chunk + small constants
            lt0 = ltp.tile([CP, gsizes[0] * BLK], f16, tag="lt")
            nc.sync.dma_start(lt0[:], lt_slice(0))
            vs_sb = cpool.tile([CP, m * NBLK], f16)
            nc.scalar.dma_start(vs_sb[:], vs_d[:])
            zt_sb = cpool.tile([BLK, NBLK], f32)
            nc.sync.dma_start(zt_sb[:], zt_d[:])
            w_sb = []
            for i in range(max(m - 1, 1)):
                w = cpool.tile([BLK, NBLK], mybir.dt.uint8, tag=f"w{i}")
                nc.scalar.dma_start(w[:], w_d[i][:])
                w_sb.append(w)

            res = [
                resp.tile([BLK, SBLK, stride], f32, tag=f"res{i}", name=f"res{i}")
                for i in range(SLICES)
            ]
            sig_full = fin.tile([BLK, NBLK], f32)
            et_full = fin.tile([BLK, NBLK], f32)
            eps_sb = fin.tile([BLK, 1], f32, tag="eps")
            nc.vector.memset(eps_sb[:], EPS)
            # numerator exp early: also triggers the exp table load off-path
            nc.scalar.activation(
                et_full[:], zt_sb[:], mybir.ActivationFunctionType.Exp
            )

            def epilogue(sl):
                cols = slice(sl * SBLK, (sl + 1) * SBLK)
                rsl = res[sl]
                dsel = fin.tile([BLK, SBLK], f32, tag="dsel")
                if m == 1:
                    nc.vector.tensor_copy(dsel[:], rsl[:, :, 0])
                else:
                    nc.vector.tensor_copy(dsel[:], rsl[:, :, m - 1])
                    for i in range(m - 2, -1, -1):
                        nc.vector.copy_predicated(
                            dsel[:], w_sb[i][:, cols], rsl[:, :, i]
                        )
                rec = fin.tile([BLK, SBLK], f32, tag="rec")
                nc.vector.reciprocal_approx_fast(rec[:], dsel[:])
                nc.vector.tensor_tensor(
                    sig_full[:, cols], et_full[:, cols], rec[:],
                    op=mybir.AluOpType.mult,
                )

            kk = 0
            done = 0
            for g, gs in enumerate(gsizes):
                if g == 0:
                    ltg = lt0
                else:
                    ltg = ltp.tile([CP, gs * BLK], f16, tag="lt")
                    eng = nc.sync if g % 2 == 0 else nc.scalar
                    eng.dma_start(ltg[:], lt_slice(g))
                na = _act_blocks(gs)
                nd = gs - na
                ca = na * BLK
                eta = etap.tile([CP, ca], f16, tag="eta")
                nc.scalar.activation(
                    eta[:], ltg[:, :ca], mybir.ActivationFunctionType.Exp,
                    scale=LN2,
                )
                etd = etdp.tile([CP, nd * BLK], f16, tag="etd")
                bits = bitp.tile([CP, nd * BLK], i32, tag="bits")
                nc.vector._custom_dve(
                    op1, out=bits[:], in0=ltg[:, ca:],
                    s0=KMAGIC, s1=P23, imm2=P23,
                )
                nc.vector._custom_dve(
                    op2, out=etd[:], in0=ltg[:, ca:],
                    in1=bits[:].bitcast(mybir.dt.float32),
                    s0=B1C, s1=B2C, imm2=KMAGIC,
                )
                if g == len(gsizes) - 1:
                    # prefetch the Ln table behind the epilogue tail
                    dummy = fin.tile([1, 1], f32, tag="dummy")
                    nc.scalar.activation(
                        dummy[:], zt_sb[0:1, 0:1], mybir.ActivationFunctionType.Ln
                    )
                for k in range(gs):
                    et = eta if k < na else etd
                    koff = k * BLK if k < na else (k - na) * BLK
                    sl, j = kk // SBLK, kk % SBLK
                    nc.tensor.matmul(
                        res[sl][:, j, 0:m],
                        et[:, koff : koff + BLK],
                        vs_sb[:, m * kk : m * (kk + 1)],
                        start=True,
                        stop=True,
                    )
                    kk += 1
                while done < SLICES and kk >= (done + 1) * SBLK:
                    epilogue(done)
                    done += 1
            while done < SLICES:
                epilogue(done)
                done += 1

            lnr = fin.tile([BLK, NBLK], f32)
            lsum = fin.tile([BLK, 1], f32)
            nc.scalar.activation(
                lnr[:],
                sig_full[:],
                mybir.ActivationFunctionType.Ln,
                bias=eps_sb[:],
                accum_out=lsum[:],
            )
            nc.sync.dma_start(out_d[:], lsum[:])

    nc.compile()
    return nc


def _pick_stride(m: int) -> int:
    for st in (1, 2, 4, 8, 16):
        if st >= m and 512 % st == 0:
            return st
    raise ValueError(f"too many classes per block: m={m}")


def kernel(logits, s, targets):
    global LAST_RESULTS
    logits = np.asarray(logits, dtype=np.float32)
    s = np.asarray(s, dtype=np.float32)
    t = np.asarray(targets).astype(np.int64).ravel()
    assert logits.shape == (B, C) and s.shape == (C, C) and t.shape == (B,)

    order = np.argsort(t, kind="stable")
    zt_all = logits[np.arange(B), t]  # host gather of logits[b, t_b]

    idxs = [order[mm::NCORES] for mm in range(NCORES)]

    m = 1
    block_classes = []
    for idx in idxs:
        tb = t[idx].reshape(NBLK, BLK)
        cs = [np.unique(row) for row in tb]
        m = max(m, max(len(u) for u in cs))
        block_classes.append((tb, cs))
    stride = _pick_stride(m)

    # column ranges (in blocks) handled by the DVE engine per chunk
    bounds = np.cumsum([0] + GSIZES)
    dve_cols = []
    for g, gs in enumerate(GSIZES):
        na = _act_blocks(gs)
        dve_cols.append(((bounds[g] + na) * BLK, bounds[g + 1] * BLK))

    in_maps = []
    for core in range(NCORES):
        idx = idxs[core]
        tb, cs = block_classes[core]
        ltT = np.empty((CP, RPC), dtype=np.float32)
        ltT[:C] = (logits[idx].T * LOG2E)
        for a, b_ in dve_cols:
            ltT[:C, a:b_] += LA
        ltT[C] = 0.0  # exp -> 1.0 row for the +eps matmul term
        ltT16 = ltT.astype(np.float16)
        cb = bounds * BLK
        lt = np.concatenate(
            [ltT16[:, a:b_].ravel() for a, b_ in zip(cb[:-1], cb[1:])]
        )
        zt = np.ascontiguousarray(zt_all[idx].reshape(NBLK, BLK).T)  # [BLK,NBLK]
        vs = np.empty((CP, m * NBLK), dtype=np.float16)
        cmat = np.empty((m, NBLK), dtype=np.int64)
        for k in range(NBLK):
            u = cs[k]
            cmat[: len(u), k] = u
            cmat[len(u) :, k] = u[-1]
        for i in range(m):
            vs[:C, i::m] = s[cmat[i]].T.astype(np.float16)
        vs[C, :] = np.float16(EPS)
        im = {"lt": lt, "vs": vs, "zt": zt}
        nw = max(m - 1, 1)
        for i in range(nw):
            wi = (tb == cmat[i][:, None]).T.astype(np.uint8)  # [BLK, NBLK]
            im[f"w{i}"] = np.ascontiguousarray(wi)
        in_maps.append(im)

    key = (m, stride)
    if key not in _nc_cache:
        _nc_cache[key] = _build_nc(m, stride)
    nc = _nc_cache[key]

    res = run_bass_kernel_spmd(
        nc, in_maps, core_ids=list(range(NCORES)), trace=TRACE
    )
    LAST_RESULTS = res
    total = sum(float(r["out"].sum(dtype=np.float64)) for r in res.results)
    return np.float32(-total / B)


# revision 8
# speedup vs baseline: 3.0904x; 3.0904x over previous
"""CPA-loss kernel for 8 TRN2 NeuronCores — v2.

Math: for row b with target t, the loss collapses to
    loss[b] = -log( e[b,t] / (dot(s[t,:], e[b,:]) + eps) + eps ),
    e = exp(z)  (max-subtraction skipped; effect ~2e-7 on the mean loss).

v2 strategy vs the 44.2us baseline (trace-driven):
 - The baseline was paced by the Scalar/ACT engine (exp of 1.64M elems/core at
   1 elem/cycle/lane) plus ~7us of gpsimd SWDGE dge_drain dead time and ~8us
   of fixed framework teardown.  Here the exp is SPLIT between the ACT engine
   and the Vector engine via two custom-DVE ops (registered at import):
     op1: bits = int32((f32(t + K) - 2^23) * 2^23)   -> raw bits of 2^round(t)
     op2: out  = (1 + b1 f + b2 f^2) * bitcast_f32(bits),  f = t - round(t)
   Host ships t = z*log2(e) (+log2(a0) on DVE-assigned columns, which folds
   the poly normalization into the data).  The DVE chain's worst-case rel
   error is 1.7e-3, mean-centered to <1e-6 bias on the final scalar (validated
   against the reference: rel err 8.5e-7 at a 50/50 split).
 - Denominator "+eps" rides the matmul: row 100 of the shipped logits is 0
   (exp -> 1.0 exactly on both engines) and row 100 of the candidate matrix is
   eps, so PSUM holds D+eps directly.
 - Epilogue: dsel select via host masks, 1/(D+eps) via the single-instruction
   reciprocal_approx_fast (~51 ULP), sigma = e_t * rec, then ONE final
   Ln(sigma + eps) with accum_out (bias=eps is free on ACT).  No gpsimd DMA.
"""

import sys

import ml_dtypes  # noqa: F401
import numpy as np

for _p in ("/opt/trn_rl_repo",):
    if _p not in sys.path:
        sys.path.append(_p)

import concourse.bass as bass  # noqa: F401
import concourse.tile as tile
from concourse import bacc, mybir
from concourse.bass_utils import run_bass_kernel_spmd
from concourse import dve_ops as _dvo
from concourse.dve_spec import Spec, Src0, Src1, C0, C1, C2, One, lower, _has_src1
from concourse.dve_uop import DveOpSpec

B = 131072
C = 100
CP = 104  # 100 classes + eps row (100) + 3 zero pad rows (DMA stripes evenly)
NCORES = 8
RPC = B // NCORES  # 16384 rows per core
BLK = 128
NBLK = RPC // BLK  # 128 blocks per core
GSIZES = [4, 8, 16, 32, 32, 36]  # blocks per DMA/exp chunk
SLICES = 4
SBLK = NBLK // SLICES
EPS = 1e-6

F = np.float32
LOG2E = float(np.log2(np.e))
LN2 = float(np.log(2.0))
KMAGIC = 8388735.0  # 2^23 + 127
P23 = 8388608.0     # 2^23
# mean-centered poly 2^f ~ a0*(1 + b1 f + b2 f^2) on [-0.5, 0.5]
B1C = 0.7031777501106262
B2C = 0.23833733797073364
LA = 0.0005543692115323172  # log2(a0), host-applied to DVE columns

# fraction of each chunk's blocks handled by the ACT engine (rest on DVE)
ACT_FRAC = 0.66


def _act_blocks(gs: int) -> int:
    return max(1, min(gs - 1, int(round(gs * ACT_FRAC))))


TRACE = False  # test.py flips this to get a profiled run
LAST_RESULTS = None  # stash of the last BassKernelResults (for test.py)

_nc_cache = {}
_ops_cache = []


def _f32(x):
    return np.float32(x)


def _ref1(in0, in1, s0, s1, imm2):
    t = in0.astype(F)
    u = (t + _f32(s0)).astype(F)
    return ((u - _f32(s1)) * _f32(imm2)).astype(F)


def _ref2(in0, in1, s0, s1, imm2):
    t = in0.astype(F)
    u = (t + _f32(imm2)).astype(F)
    n = (u - _f32(imm2)).astype(F)
    f = (t - n).astype(F)
    q = ((_f32(s1) * f).astype(F) + _f32(s0)).astype(F)
    q = (q * f).astype(F)
    q = (q + _f32(1.0)).astype(F)
    return (q * in1.astype(F)).astype(F)


def _register_dve_ops():
    """Register the two exp2 custom-DVE ops (idempotent)."""
    global _ops_cache
    if _ops_cache:
        return _ops_cache
    if "EXP2_BITS_ANT" in _dvo._SUB_OPCODE_FOR_NAME:
        by_name = {o.name: o for o in _dvo.OPS}
        _ops_cache = [by_name["EXP2_BITS_ANT"], by_name["EXP2_FIN_ANT"]]
        return _ops_cache

    def mk(name, body, ref):
        opcode = _dvo._CUSTOM_DVE_ROW_BASE + len(_dvo.OPS)
        spec = Spec(body=body, reference=ref)
        shas = {}
        for ver in ("v3", "v4"):
            ds = DveOpSpec(
                name=name, opcode=opcode, uops=lower(spec, ver=ver),
                rd1_en=_has_src1(spec),
            )
            shas[ver] = ds.sha(ver)
        op = _dvo.DveOp(name, spec, subdim=False, uops_sha=shas)
        _dvo.OPS.append(op)
        _dvo._SUB_OPCODE_FOR_NAME[name] = opcode
        _dvo.CUSTOM_DVE_SPECS[name] = op.spec
        return op

    op1 = mk("EXP2_BITS_ANT", ((Src0 + C0) - C1) * C2, _ref1)
    _u = Src0 + C2
    _n = _u - C2
    _fr = Src0 - _n
    _q = ((C1 * _fr) + C0) * _fr + One
    op2 = mk("EXP2_FIN_ANT", _q * Src1, _ref2)
    _ops_cache = [op1, op2]
    return _ops_cache


def _build_nc(m: int, stride: int):
    op1, op2 = _register_dve_ops()
    nc = bacc.Bacc("TRN2", target_bir_lowering=False, debug=False)
    f32 = mybir.dt.float32
    f16 = mybir.dt.float16
    i32 = mybir.dt.int32

    lt_d = nc.declare_dram_parameter("lt", [CP * RPC], f16, isOutput=False)
    vs_d = nc.declare_dram_parameter("vs", [CP, m * NBLK], f16, isOutput=False)
    zt_d = nc.declare_dram_parameter("zt", [BLK, NBLK], f32, isOutput=False)
    w_d = [
        nc.declare_dram_parameter(f"w{i}", [BLK, NBLK], mybir.dt.uint8, isOutput=False)
        for i in range(max(m - 1, 1))
    ]
    out_d = nc.declare_dram_parameter("out", [BLK, 1], f32, isOutput=True)

    gsizes = GSIZES
    assert sum(gsizes) == NBLK

    with tile.TileContext(nc) as tc:
        with (
            tc.tile_pool(name="const", bufs=1) as cpool,
            tc.tile_pool(name="eta", bufs=2) as etap,
            tc.tile_pool(name="etd", bufs=2) as etdp,
            tc.tile_pool(name="bits", bufs=2) as bitp,
            tc.tile_pool(name="fin", bufs=1) as fin,
            tc.tile_pool(name="res", bufs=1, space="PSUM") as resp,
        ):
            def lt_slice(g):
                off = CP * BLK * sum(gsizes[:g])
                n = CP * gsizes[g] * BLK
                return lt_d[off : off + n].rearrange("(j c) -> j c", j=CP)

            # static lt chunk tiles; all DMA triggers issue up front so both
            # HWDGE queues stream back-to-back (scalar's triggers precede all
            # ACT compute).  sync: g0..g3 + consts; scalar: g4, g5.
            lt_t = [
                cpool.tile([CP, gs * BLK], f16, tag=f"ltg{g}", name=f"ltg{g}")
                for g, gs in enumerate(gsizes)
            ]
            nc.sync.dma_start(lt_t[0][:], lt_slice(0))
            nc.scalar.dma_start(lt_t[4][:], lt_slice(4))
            nc.scalar.dma_start(lt_t[5][:], lt_slice(5))
            zt_sb = cpool.tile([BLK, NBLK], f32)
            nc.sync.dma_start(zt_sb[:], zt_d[:])
            vs_sb = cpool.tile([CP, m * NBLK], f16)
            nc.sync.dma_start(vs_sb[:], vs_d[:])
            nc.sync.dma_start(lt_t[1][:], lt_slice(1))
            w_sb = []
            for i in range(max(m - 1, 1)):
                w = cpool.tile([BLK, NBLK], mybir.dt.uint8, tag=f"w{i}")
                nc.sync.dma_start(w[:], w_d[i][:])
                w_sb.append(w)
            nc.sync.dma_start(lt_t[2][:], lt_slice(2))
            nc.sync.dma_start(lt_t[3][:], lt_slice(3))

            res = [
                resp.tile([BLK, SBLK, stride], f32, tag=f"res{i}", name=f"res{i}")
                for i in range(SLICES)
            ]
            sig_full = fin.tile([BLK, NBLK], f32)
            et_full = fin.tile([BLK, NBLK], f32)
            eps_sb = fin.tile([BLK, 1], f32, tag="eps")
            nc.vector.memset(eps_sb[:], EPS)
            # numerator exp early: also triggers the exp table load off-path
            nc.scalar.activation(
                et_full[:], zt_sb[:], mybir.ActivationFunctionType.Exp
            )

            def epilogue(sl):
                cols = slice(sl * SBLK, (sl + 1) * SBLK)
                rsl = res[sl]
                dsel = fin.tile([BLK, SBLK], f32, tag="dsel")
                if m == 1:
                    nc.vector.tensor_copy(dsel[:], rsl[:, :, 0])
                else:
                    nc.vector.tensor_copy(dsel[:], rsl[:, :, m - 1])
                    for i in range(m - 2, -1, -1):
                        nc.vector.copy_predicated(
                            dsel[:], w_sb[i][:, cols], rsl[:, :, i]
                        )
                rec = fin.tile([BLK, SBLK], f32, tag="rec")
                nc.vector.reciprocal_approx_fast(rec[:], dsel[:])
                nc.vector.tensor_tensor(
                    sig_full[:, cols], et_full[:, cols], rec[:],
                    op=mybir.AluOpType.mult,
                )

            kk = 0
            done = 0
            for g, gs in enumerate(gsizes):
                ltg = lt_t[g]
                na = _act_blocks(gs)
                nd = gs - na
                ca = na * BLK
                eta = etap.tile([CP, ca], f16, tag="eta")
                nc.scalar.activation(
                    eta[:], ltg[:, :ca], mybir.ActivationFunctionType.Exp,
                    scale=LN2,
                )
                etd = etdp.tile([CP, nd * BLK], f16, tag="etd")
                bits = bitp.tile([CP, nd * BLK], i32, tag="bits")
                nc.vector._custom_dve(
                    op1, out=bits[:], in0=ltg[:, ca:],
                    s0=KMAGIC, s1=P23, imm2=P23,
                )
                nc.vector._custom_dve(
                    op2, out=etd[:], in0=ltg[:, ca:],
                    in1=bits[:].bitcast(mybir.dt.float32),
                    s0=B1C, s1=B2C, imm2=KMAGIC,
                )
                if g == len(gsizes) - 1:
                    # prefetch the Ln table behind the epilogue tail
                    dummy = fin.tile([1, 1], f32, tag="dummy")
                    nc.scalar.activation(
                        dummy[:], zt_sb[0:1, 0:1], mybir.ActivationFunctionType.Ln
                    )
                for k in range(gs):
                    et = eta if k < na else etd
                    koff = k * BLK if k < na else (k - na) * BLK
                    sl, j = kk // SBLK, kk % SBLK
                    nc.tensor.matmul(
                        res[sl][:, j, 0:m],
                        et[:, koff : koff + BLK],
                        vs_sb[:, m * kk : m * (kk + 1)],
                        start=True,
                        stop=True,
                    )
                    kk += 1
                while done < SLICES and kk >= (done + 1) * SBLK:
                    epilogue(done)
                    done += 1
            while done < SLICES:
                epilogue(done)
                done += 1

            lnr = fin.tile([BLK, NBLK], f32)
            lsum = fin.tile([BLK, 1], f32)
            nc.scalar.activation(
                lnr[:],
                sig_full[:],
                mybir.ActivationFunctionType.Ln,
                bias=eps_sb[:],
                accum_out=lsum[:],
            )
            nc.sync.dma_start(out_d[:], lsum[:])

    nc.compile()
    return nc


def _pick_stride(m: int) -> int:
    for st in (1, 2, 4, 8, 16):
        if st >= m and 512 % st == 0:
            return st
    raise ValueError(f"too many classes per block: m={m}")


def kernel(logits, s, targets):
    global LAST_RESULTS
    logits = np.asarray(logits, dtype=np.float32)
    s = np.asarray(s, dtype=np.float32)
    t = np.asarray(targets).astype(np.int64).ravel()
    assert logits.shape == (B, C) and s.shape == (C, C) and t.shape == (B,)

    order = np.argsort(t, kind="stable")
    zt_all = logits[np.arange(B), t]  # host gather of logits[b, t_b]

    idxs = [order[mm::NCORES] for mm in range(NCORES)]

    m = 1
    block_classes = []
    for idx in idxs:
        tb = t[idx].reshape(NBLK, BLK)
        cs = [np.unique(row) for row in tb]
        m = max(m, max(len(u) for u in cs))
        block_classes.append((tb, cs))
    stride = _pick_stride(m)

    # column ranges (in blocks) handled by the DVE engine per chunk
    bounds = np.cumsum([0] + GSIZES)
    dve_cols = []
    for g, gs in enumerate(GSIZES):
        na = _act_blocks(gs)
        dve_cols.append(((bounds[g] + na) * BLK, bounds[g + 1] * BLK))

    in_maps = []
    for core in range(NCORES):
        idx = idxs[core]
        tb, cs = block_classes[core]
        ltT = np.zeros((CP, RPC), dtype=np.float32)
        ltT[:C] = (logits[idx].T * LOG2E)
        for a, b_ in dve_cols:
            ltT[:C, a:b_] += LA
        # rows C..CP-1 stay 0 (exp -> 1.0); only row C gets eps weight in vs
        ltT16 = ltT.astype(np.float16)
        cb = bounds * BLK
        lt = np.concatenate(
            [ltT16[:, a:b_].ravel() for a, b_ in zip(cb[:-1], cb[1:])]
        )
        zt = np.ascontiguousarray(zt_all[idx].reshape(NBLK, BLK).T)  # [BLK,NBLK]
        vs = np.zeros((CP, m * NBLK), dtype=np.float16)
        cmat = np.empty((m, NBLK), dtype=np.int64)
        for k in range(NBLK):
            u = cs[k]
            cmat[: len(u), k] = u
            cmat[len(u) :, k] = u[-1]
        for i in range(m):
            vs[:C, i::m] = s[cmat[i]].T.astype(np.float16)
        vs[C, :] = np.float16(EPS)
        im = {"lt": lt, "vs": vs, "zt": zt}
        nw = max(m - 1, 1)
        for i in range(nw):
            wi = (tb == cmat[i][:, None]).T.astype(np.uint8)  # [BLK, NBLK]
            im[f"w{i}"] = np.ascontiguousarray(wi)
        in_maps.append(im)

    key = (m, stride)
    if key not in _nc_cache:
        _nc_cache[key] = _build_nc(m, stride)
    nc = _nc_cache[key]

    res = run_bass_kernel_spmd(
        nc, in_maps, core_ids=list(range(NCORES)), trace=TRACE
    )
    LAST_RESULTS = res
    total = sum(float(r["out"].sum(dtype=np.float64)) for r in res.results)
    return np.float32(-total / B)


# revision 9
# speedup vs baseline: 3.4551x; 1.1180x over previous
"""CPA-loss kernel for 8 TRN2 NeuronCores — v2.

Math: for row b with target t, the loss collapses to
    loss[b] = -log( e[b,t] / (dot(s[t,:], e[b,:]) + eps) + eps ),
    e = exp(z)  (max-subtraction skipped; effect ~2e-7 on the mean loss).

v2 strategy vs the 44.2us baseline (trace-driven):
 - The baseline was paced by the Scalar/ACT engine (exp of 1.64M elems/core at
   1 elem/cycle/lane) plus ~7us of gpsimd SWDGE dge_drain dead time and ~8us
   of fixed framework teardown.  Here the exp is SPLIT between the ACT engine
   and the Vector engine via two custom-DVE ops (registered at import):
     op1: bits = int32((f32(t + K) - 2^23) * 2^23)   -> raw bits of 2^round(t)
     op2: out  = (1 + b1 f + b2 f^2) * bitcast_f32(bits),  f = t - round(t)
   Host ships t = z*log2(e) (+log2(a0) on DVE-assigned columns, which folds
   the poly normalization into the data).  The DVE chain's worst-case rel
   error is 1.7e-3, mean-centered to <1e-6 bias on the final scalar (validated
   against the reference: rel err 8.5e-7 at a 50/50 split).
 - Denominator "+eps" rides the matmul: row 100 of the shipped logits is 0
   (exp -> 1.0 exactly on both engines) and row 100 of the candidate matrix is
   eps, so PSUM holds D+eps directly.
 - Epilogue: dsel select via host masks, 1/(D+eps) via the single-instruction
   reciprocal_approx_fast (~51 ULP), sigma = e_t * rec, then ONE final
   Ln(sigma + eps) with accum_out (bias=eps is free on ACT).  No gpsimd DMA.
"""

import sys

import ml_dtypes  # noqa: F401
import numpy as np

for _p in ("/opt/trn_rl_repo",):
    if _p not in sys.path:
        sys.path.append(_p)

import concourse.bass as bass  # noqa: F401
import concourse.tile as tile
from concourse import bacc, mybir
from concourse.bass_utils import run_bass_kernel_spmd
from concourse import dve_ops as _dvo
from concourse.dve_spec import Spec, Src0, Src1, C0, C1, C2, One, lower, _has_src1
from concourse.dve_uop import DveOpSpec

B = 131072
C = 100
CP = 104  # 100 classes + eps row (100) + 3 zero pad rows (DMA stripes evenly)
NCORES = 8
RPC = B // NCORES  # 16384 rows per core
BLK = 128
NBLK = RPC // BLK  # 128 blocks per core
GSIZES = [4, 8, 16, 32, 32, 36]  # blocks per DMA/exp chunk
SLICES = 4
SBLK = NBLK // SLICES
EPS = 1e-6

F = np.float32
LOG2E = float(np.log2(np.e))
LN2 = float(np.log(2.0))
KMAGIC = 8388735.0  # 2^23 + 127
P23 = 8388608.0     # 2^23
# mean-centered poly 2^f ~ a0*(1 + b1 f + b2 f^2) on [-0.5, 0.5]
B1C = 0.7031777501106262
B2C = 0.23833733797073364
LA = 0.0005543692115323172  # log2(a0), host-applied to DVE columns

# fraction of each chunk's blocks handled by the ACT engine (rest on DVE)
ACT_FRAC = 0.66


def _act_blocks(gs: int) -> int:
    return max(1, min(gs - 1, int(round(gs * ACT_FRAC))))


TRACE = False  # test.py flips this to get a profiled run
LAST_RESULTS = None  # stash of the last BassKernelResults (for test.py)

_nc_cache = {}
_ops_cache = []


def _f32(x):
    return np.float32(x)


def _ref1(in0, in1, s0, s1, imm2):
    t = in0.astype(F)
    u = (t + _f32(s0)).astype(F)
    return ((u - _f32(s1)) * _f32(imm2)).astype(F)


def _ref2(in0, in1, s0, s1, imm2):
    t = in0.astype(F)
    u = (t + _f32(imm2)).astype(F)
    n = (u - _f32(imm2)).astype(F)
    f = (t - n).astype(F)
    q = ((_f32(s1) * f).astype(F) + _f32(s0)).astype(F)
    q = (q * f).astype(F)
    q = (q + _f32(1.0)).astype(F)
    return (q * in1.astype(F)).astype(F)


def _register_dve_ops():
    """Register the two exp2 custom-DVE ops (idempotent)."""
    global _ops_cache
    if _ops_cache:
        return _ops_cache
    if "EXP2_BITS_ANT" in _dvo._SUB_OPCODE_FOR_NAME:
        by_name = {o.name: o for o in _dvo.OPS}
        _ops_cache = [by_name["EXP2_BITS_ANT"], by_name["EXP2_FIN_ANT"]]
        return _ops_cache

    def mk(name, body, ref):
        opcode = _dvo._CUSTOM_DVE_ROW_BASE + len(_dvo.OPS)
        spec = Spec(body=body, reference=ref)
        shas = {}
        for ver in ("v3", "v4"):
            ds = DveOpSpec(
                name=name, opcode=opcode, uops=lower(spec, ver=ver),
                rd1_en=_has_src1(spec),
            )
            shas[ver] = ds.sha(ver)
        op = _dvo.DveOp(name, spec, subdim=False, uops_sha=shas)
        _dvo.OPS.append(op)
        _dvo._SUB_OPCODE_FOR_NAME[name] = opcode
        _dvo.CUSTOM_DVE_SPECS[name] = op.spec
        return op

    op1 = mk("EXP2_BITS_ANT", ((Src0 + C0) - C1) * C2, _ref1)
    _u = Src0 + C2
    _n = _u - C2
    _fr = Src0 - _n
    _q = ((C1 * _fr) + C0) * _fr + One
    op2 = mk("EXP2_FIN_ANT", _q * Src1, _ref2)
    _ops_cache = [op1, op2]
    return _ops_cache


def _build_nc(m: int, stride: int):
    op1, op2 = _register_dve_ops()
    nc = bacc.Bacc("TRN2", target_bir_lowering=False, debug=False)
    f32 = mybir.dt.float32
    f16 = mybir.dt.float16
    i32 = mybir.dt.int32

    lt_d = nc.declare_dram_parameter("lt", [CP * RPC], f16, isOutput=False)
    vs_d = nc.declare_dram_parameter("vs", [CP, m * NBLK], f16, isOutput=False)
    zt_d = nc.declare_dram_parameter("zt", [BLK, NBLK], f32, isOutput=False)
    w_d = [
        nc.declare_dram_parameter(f"w{i}", [BLK, NBLK], mybir.dt.uint8, isOutput=False)
        for i in range(max(m - 1, 1))
    ]
    out_d = nc.declare_dram_parameter("out", [BLK, 1], f32, isOutput=True)

    gsizes = GSIZES
    assert sum(gsizes) == NBLK

    with tile.TileContext(nc) as tc:
        with (
            tc.tile_pool(name="const", bufs=1) as cpool,
            tc.tile_pool(name="eta", bufs=2) as etap,
            tc.tile_pool(name="etd", bufs=2) as etdp,
            tc.tile_pool(name="bits", bufs=2) as bitp,
            tc.tile_pool(name="fin", bufs=1) as fin,
            tc.tile_pool(name="res", bufs=1, space="PSUM") as resp,
        ):
            def lt_slice(g):
                off = CP * BLK * sum(gsizes[:g])
                n = CP * gsizes[g] * BLK
                return lt_d[off : off + n].rearrange("(j c) -> j c", j=CP)

            # static lt chunk tiles; all DMA triggers issue up front so both
            # HWDGE queues stream back-to-back (scalar's triggers precede all
            # ACT compute).  sync: g0..g3 + consts; scalar: g4, g5.
            lt_t = [
                cpool.tile([CP, gs * BLK], f16, tag=f"ltg{g}", name=f"ltg{g}")
                for g, gs in enumerate(gsizes)
            ]
            nc.sync.dma_start(lt_t[0][:], lt_slice(0))
            nc.scalar.dma_start(lt_t[1][:], lt_slice(1))
            zt_sb = cpool.tile([BLK, NBLK], f32)
            nc.sync.dma_start(zt_sb[:], zt_d[:])
            vs_sb = cpool.tile([CP, m * NBLK], f16)
            nc.sync.dma_start(vs_sb[:], vs_d[:])
            w_sb = []
            for i in range(max(m - 1, 1)):
                w = cpool.tile([BLK, NBLK], mybir.dt.uint8, tag=f"w{i}")
                nc.sync.dma_start(w[:], w_d[i][:])
                w_sb.append(w)
            nc.sync.dma_start(lt_t[2][:], lt_slice(2))
            nc.sync.dma_start(lt_t[4][:], lt_slice(4))

            res = [
                resp.tile([BLK, SBLK, stride], f32, tag=f"res{i}", name=f"res{i}")
                for i in range(SLICES)
            ]
            sig_full = fin.tile([BLK, NBLK], f32)
            et_full = fin.tile([BLK, NBLK], f32)
            eps_sb = fin.tile([BLK, 1], f32, tag="eps")
            nc.vector.memset(eps_sb[:], EPS)
            # numerator exp early: also triggers the exp table load off-path
            nc.scalar.activation(
                et_full[:], zt_sb[:], mybir.ActivationFunctionType.Exp
            )

            def epilogue(sl):
                cols = slice(sl * SBLK, (sl + 1) * SBLK)
                rsl = res[sl]
                dsel = fin.tile([BLK, SBLK], f32, tag="dsel")
                if m == 1:
                    nc.vector.tensor_copy(dsel[:], rsl[:, :, 0])
                else:
                    nc.vector.tensor_copy(dsel[:], rsl[:, :, m - 1])
                    for i in range(m - 2, -1, -1):
                        nc.vector.copy_predicated(
                            dsel[:], w_sb[i][:, cols], rsl[:, :, i]
                        )
                rec = fin.tile([BLK, SBLK], f32, tag="rec")
                nc.vector.reciprocal_approx_fast(rec[:], dsel[:])
                nc.vector.tensor_tensor(
                    sig_full[:, cols], et_full[:, cols], rec[:],
                    op=mybir.AluOpType.mult,
                )

            kk = 0
            done = 0
            for g, gs in enumerate(gsizes):
                ltg = lt_t[g]
                na = _act_blocks(gs)
                nd = gs - na
                ca = na * BLK
                eta = etap.tile([CP, ca], f16, tag="eta")
                nc.scalar.activation(
                    eta[:], ltg[:, :ca], mybir.ActivationFunctionType.Exp,
                    scale=LN2,
                )
                etd = etdp.tile([CP, nd * BLK], f16, tag="etd")
                bits = bitp.tile([CP, nd * BLK], i32, tag="bits")
                nc.vector._custom_dve(
                    op1, out=bits[:], in0=ltg[:, ca:],
                    s0=KMAGIC, s1=P23, imm2=P23,
                )
                nc.vector._custom_dve(
                    op2, out=etd[:], in0=ltg[:, ca:],
                    in1=bits[:].bitcast(mybir.dt.float32),
                    s0=B1C, s1=B2C, imm2=KMAGIC,
                )
                if g == 1:
                    nc.scalar.dma_start(lt_t[3][:], lt_slice(3))
                if g == 2:
                    nc.scalar.dma_start(lt_t[5][:], lt_slice(5))
                if g == len(gsizes) - 1:
                    # prefetch the Ln table behind the epilogue tail
                    dummy = fin.tile([1, 1], f32, tag="dummy")
                    nc.scalar.activation(
                        dummy[:], zt_sb[0:1, 0:1], mybir.ActivationFunctionType.Ln
                    )
                for k in range(gs):
                    et = eta if k < na else etd
                    koff = k * BLK if k < na else (k - na) * BLK
                    sl, j = kk // SBLK, kk % SBLK
                    nc.tensor.matmul(
                        res[sl][:, j, 0:m],
                        et[:, koff : koff + BLK],
                        vs_sb[:, m * kk : m * (kk + 1)],
                        start=True,
                        stop=True,
                    )
                    kk += 1
                while done < SLICES and kk >= (done + 1) * SBLK:
                    epilogue(done)
                    done += 1
            while done < SLICES:
                epilogue(done)
                done += 1

            lnr = fin.tile([BLK, NBLK], f32)
            lsum = fin.tile([BLK, 1], f32)
            nc.scalar.activation(
                lnr[:],
                sig_full[:],
                mybir.ActivationFunctionType.Ln,
                bias=eps_sb[:],
                accum_out=lsum[:],
            )
            nc.sync.dma_start(out_d[:], lsum[:])

    nc.compile()
    return nc


def _pick_stride(m: int) -> int:
    for st in (1, 2, 4, 8, 16):
        if st >= m and 512 % st == 0:
            return st
    raise ValueError(f"too many classes per block: m={m}")


def kernel(logits, s, targets):
    global LAST_RESULTS
    logits = np.asarray(logits, dtype=np.float32)
    s = np.asarray(s, dtype=np.float32)
    t = np.asarray(targets).astype(np.int64).ravel()
    assert logits.shape == (B, C) and s.shape == (C, C) and t.shape == (B,)

    order = np.argsort(t, kind="stable")
    zt_all = logits[np.arange(B), t]  # host gather of logits[b, t_b]

    idxs = [order[mm::NCORES] for mm in range(NCORES)]

    m = 1
    block_classes = []
    for idx in idxs:
        tb = t[idx].reshape(NBLK, BLK)
        cs = [np.unique(row) for row in tb]
        m = max(m, max(len(u) for u in cs))
        block_classes.append((tb, cs))
    stride = _pick_stride(m)

    # column ranges (in blocks) handled by the DVE engine per chunk
    bounds = np.cumsum([0] + GSIZES)
    dve_cols = []
    for g, gs in enumerate(GSIZES):
        na = _act_blocks(gs)
        dve_cols.append(((bounds[g] + na) * BLK, bounds[g + 1] * BLK))

    in_maps = []
    for core in range(NCORES):
        idx = idxs[core]
        tb, cs = block_classes[core]
        ltT = np.zeros((CP, RPC), dtype=np.float32)
        ltT[:C] = (logits[idx].T * LOG2E)
        for a, b_ in dve_cols:
            ltT[:C, a:b_] += LA
        # rows C..CP-1 stay 0 (exp -> 1.0); only row C gets eps weight in vs
        ltT16 = ltT.astype(np.float16)
        cb = bounds * BLK
        lt = np.concatenate(
            [ltT16[:, a:b_].ravel() for a, b_ in zip(cb[:-1], cb[1:])]
        )
        zt = np.ascontiguousarray(zt_all[idx].reshape(NBLK, BLK).T)  # [BLK,NBLK]
        vs = np.zeros((CP, m * NBLK), dtype=np.float16)
        cmat = np.empty((m, NBLK), dtype=np.int64)
        for k in range(NBLK):
            u = cs[k]
            cmat[: len(u), k] = u
            cmat[len(u) :, k] = u[-1]
        for i in range(m):
            vs[:C, i::m] = s[cmat[i]].T.astype(np.float16)
        vs[C, :] = np.float16(EPS)
        im = {"lt": lt, "vs": vs, "zt": zt}
        nw = max(m - 1, 1)
        for i in range(nw):
            wi = (tb == cmat[i][:, None]).T.astype(np.uint8)  # [BLK, NBLK]
            im[f"w{i}"] = np.ascontiguousarray(wi)
        in_maps.append(im)

    key = (m, stride)
    if key not in _nc_cache:
        _nc_cache[key] = _build_nc(m, stride)
    nc = _nc_cache[key]

    res = run_bass_kernel_spmd(
        nc, in_maps, core_ids=list(range(NCORES)), trace=TRACE
    )
    LAST_RESULTS = res
    total = sum(float(r["out"].sum(dtype=np.float64)) for r in res.results)
    return np.float32(-total / B)
